# revision 1
# baseline (speedup 1.0000x reference)
"""nn_Lookahead v9: flipped matmul (x stationary, bands moving), D=128.

Flip rationale: stationary loads are free in the cost model, so putting the
x time-tile slabs in the PE array and streaming the small band blocks as
the moving operand cuts PE to ~21us. That makes stride-128 tiles viable
again (no x overlap: -4.4us DMA) despite the spill matmul, since PE has
huge slack. Bands revert to the 3-region staging (A/B/C) at +1.8us.
DMA busy: x 23.3 + bands 7.65 + y 11.67 = 42.6us vs 45.2 for v8.

Per feature f, i-block j (8 tiles = 128 stationary columns):
  mA: psum[(i,b), tau 0:64]    = x[0:84, blk]^T   . bandA[0:84, 64]
  mB: psum[(i,b), tau 64:128]  = x[64:128, blk]^T . bandB[64:128@p64, 64]
  mC: psum[(ib<112), tau 64:128]+= x_next[0:20, blk+1]^T . bandC[0:20, 64]
band84[a,t] = w[f, a-t]*YGAIN (0<=a-t<=20); A = band84[0:84],
B = band84[0:64] restaged at p64..128, C = band84[64:84] at p0..20.
"""

import sys

sys.path.insert(0, "/opt/trn_rl_repo")

import numpy as np

T, B, F, K = 2048, 16, 1024, 21
YGAIN = 127.0 / 4.5
CTX = K - 1
NCORES = 8
FC = F // NCORES
S = 128            # time-tile size = stride (no overlap)
NI = T // S        # 16 tiles
NIB = NI * B       # 256 x-columns per feature
NBLK = 2           # i-blocks per feature (8 tiles = 128 stationary cols)
BLKC = NIB // NBLK  # 128
W64 = 64
AH = W64 + CTX     # 84
SB_B = W64         # bandB rows
CHUNKS = (12, 16, 24, 24, 24, 16, 8, 4)
YS = 2
HOLD_AT = 0
HOLD_CHUNKS = 4
HOLD_PRE = 5

assert sum(CHUNKS) == FC

_MODULE_CACHE = {}


def _offsets():
    xo, bo, yo = [], [], []
    brows = AH + SB_B + CTX   # 168 band rows per feature, 64 cols each
    x_acc = b_acc = y_acc = 0
    for fq in CHUNKS:
        xo.append(x_acc); x_acc += S * fq * NIB
        bo.append(b_acc); b_acc += brows * W64 * fq
        yo.append(y_acc); y_acc += S * fq * NIB
    return xo, bo, yo, x_acc, b_acc, y_acc


def build_module(repeat=1, bufs=(5, 3, 5, 8)):
    key = ("nc", repeat, bufs)
    if key in _MODULE_CACHE:
        return _MODULE_CACHE[key]
    import concourse.bacc as bacc
    import concourse.mybir as mybir
    from concourse.tile import TileContext

    xb, bb_, yb, pb = bufs
    dt = mybir.dt.float16
    nc = bacc.Bacc("TRN2", target_bir_lowering=False, debug=False,
                   num_devices=NCORES)

    xo, bo, yo, xn, bn, yn = _offsets()
    x_d = nc.dram_tensor("x", [xn], dt, kind="ExternalInput")
    b_d = nc.dram_tensor("bands", [bn], dt, kind="ExternalInput")
    y_d = nc.dram_tensor("y", [yn], mybir.dt.int8, kind="ExternalOutput")

    with TileContext(nc) as tc:
        with tc.tile_pool(name="xp", bufs=xb) as xp, \
             tc.tile_pool(name="bp", bufs=bb_) as bp, \
             tc.tile_pool(name="yp", bufs=yb) as yp, \
             tc.tile_pool(name="yh", bufs=2 * HOLD_CHUNKS) as yh, \
             tc.tile_pool(name="pp", bufs=pb, space="PSUM") as pp:
            for _ in range(repeat):
                held = []
                for ci, fq in enumerate(CHUNKS):
                    if ci == len(CHUNKS) - 1 and HOLD_PRE and held:
                        for hdst, hsb in held[:HOLD_PRE]:
                            nc.sync.dma_start(out=hdst, in_=hsb[:])
                        held = held[HOLD_PRE:]
                    fq2 = fq // YS
                    r1 = fq * W64
                    xq = xp.tile([S, fq * NIB], dt, tag="x")
                    bb = bp.tile([S, 2 * fq * W64], dt, tag="bb")

                    x_src = x_d.ap()[xo[ci]:xo[ci] + S * fq * NIB] \
                        .rearrange("(s m) -> s m", s=S, m=fq * NIB)
                    nc.sync.dma_start(out=xq[:], in_=x_src)

                    ba = bo[ci]
                    a_n, b_n, c_n = AH * r1, SB_B * r1, CTX * r1
                    a_src = b_d.ap()[ba:ba + a_n] \
                        .rearrange("(a m) -> a m", a=AH, m=r1)
                    nc.sync.dma_start(out=bb[0:AH, 0:r1], in_=a_src)
                    b_src = b_d.ap()[ba + a_n:ba + a_n + b_n] \
                        .rearrange("(a m) -> a m", a=SB_B, m=r1)
                    nc.sync.dma_start(out=bb[W64:S, r1:2 * r1], in_=b_src)
                    c_src = b_d.ap()[ba + a_n + b_n:ba + a_n + b_n + c_n] \
                        .rearrange("(a m) -> a m", a=CTX, m=r1)
                    nc.sync.dma_start(out=bb[0:CTX, r1:2 * r1], in_=c_src)

                    last = ci == len(CHUNKS) - 1
                    ysb = None
                    for fi in range(fq):
                        if last and fi == HOLD_AT and held:
                            for hdst, hsb in held:
                                nc.sync.dma_start(out=hdst, in_=hsb[:])
                            held = []
                        if last:
                            if fi == 0:
                                ysb = yp.tile([S, fq * NIB], mybir.dt.int8,
                                              tag="y")
                        elif fi % fq2 == 0:
                            if ci < HOLD_CHUNKS:
                                ysb = yh.tile([S, fq2 * NIB], mybir.dt.int8,
                                              tag="yh")
                            else:
                                ysb = yp.tile([S, fq2 * NIB], mybir.dt.int8,
                                              tag="y")
                        pt = pp.tile([S, NIB], mybir.dt.float32, tag="ps")
                        wa = fi * W64
                        for j in range(NBLK):
                            cb = j * BLKC
                            xw = fi * NIB + j * BLKC
                            # mA: stationary x rows 0:84, moving bandA.
                            nc.tensor.matmul(
                                pt[0:S, cb:cb + W64],
                                lhsT=xq[0:AH, xw:xw + BLKC],
                                rhs=bb[0:AH, wa:wa + W64],
                                start=True, stop=True, skip_group_check=True)
                            # mB: stationary x rows 64:128, moving bandB.
                            nc.tensor.matmul(
                                pt[0:S, cb + W64:cb + BLKC],
                                lhsT=xq[W64:S, xw:xw + BLKC],
                                rhs=bb[W64:S, r1 + wa:r1 + wa + W64],
                                start=True, stop=False,
                                skip_group_check=True)
                            # mC: next-tile spill; block 1 drops tile 15
                            # (zero tail padding -> 7-tile stationary).
                            nc2 = BLKC if j == 0 else BLKC - B
                            nc.tensor.matmul(
                                pt[0:nc2, cb + W64:cb + BLKC],
                                lhsT=xq[0:CTX, xw + B:xw + B + nc2],
                                rhs=bb[0:CTX, r1 + wa:r1 + wa + W64],
                                start=False, stop=True,
                                skip_group_check=True)
                        fl = fi if last else fi % fq2
                        nhalf = fq if last else fq2
                        yc = fl * NIB
                        if (nhalf - 1 - fl) % 2 == 1:
                            nc.vector.tensor_copy(ysb[:, yc:yc + NIB],
                                                  pt[:, :])
                        else:
                            nc.scalar.copy(ysb[:, yc:yc + NIB], pt[:, :])
                        if not last and fi % fq2 == fq2 - 1:
                            h = fi // fq2
                            dst = y_d.ap()[yo[ci] + h * S * fq2 * NIB:
                                           yo[ci] + (h + 1) * S * fq2 * NIB] \
                                .rearrange("(s m) -> s m", s=S, m=fq2 * NIB)
                            if ci < HOLD_CHUNKS:
                                held.append((dst, ysb))
                            else:
                                nc.scalar.dma_start(out=dst, in_=ysb[:])
                    if last:
                        dst2 = y_d.ap()[yo[ci]:yo[ci] + S * fq * NIB] \
                            .rearrange("(s m) -> s m", s=S, m=fq * NIB)
                        nc.sync.dma_start(out=dst2, in_=ysb[:])
                for dst, ysb in held:
                    nc.scalar.dma_start(out=dst, in_=ysb[:])

    nc.compile()
    _MODULE_CACHE[key] = nc
    return nc


def prep_x(x):
    """x (T,B,F) -> per-core flat fp16 [s, f, i, b], 16 non-overlap tiles."""
    xr = np.asarray(x, dtype=np.float32).reshape(NI, S, B, NCORES, FC)
    out = []
    for c in range(NCORES):
        parts = []
        f0 = 0
        for fq in CHUNKS:
            blk = xr[:, :, :, c, f0:f0 + fq]          # (i, s, b, f)
            parts.append(np.ascontiguousarray(
                blk.transpose(1, 3, 0, 2)).ravel())   # (s, f, i, b)
            f0 += fq
        out.append(np.concatenate(parts).astype(np.float16))
    return np.stack(out)


def prep_bands(weight):
    """weight (F,21) -> per-core flat band regions A/B/C, (a, f, t)."""
    w = np.asarray(weight, dtype=np.float32).reshape(NCORES, FC, K) * YGAIN
    band = np.zeros((NCORES, AH, FC, W64), np.float32)
    for k in range(K):
        for tt in range(W64):
            band[:, tt + k, :, tt] = w[:, :, k]
    out = []
    for c in range(NCORES):
        parts = []
        f0 = 0
        for fq in CHUNKS:
            blk = band[c, :, f0:f0 + fq, :]
            parts.append(blk[0:AH].ravel())
            parts.append(blk[0:SB_B].ravel())
            parts.append(blk[SB_B:AH].ravel())
            f0 += fq
        out.append(np.concatenate(parts).astype(np.float16))
    return np.stack(out)


def assemble_y(shards):
    """per-core flat int8 y [(i_loc,b), (f, j, tau)] -> (T,B,F) fp32."""
    y = np.empty((NBLK, NI // NBLK, S, B, NCORES, FC), np.float32)
    for c in range(NCORES):
        flat = np.asarray(shards[c]).astype(np.float32).ravel() / YGAIN
        f0 = 0
        o = 0
        for ci, fq in enumerate(CHUNKS):
            lastc = ci == len(CHUNKS) - 1
            nst = 1 if lastc else YS
            fqs = fq if lastc else fq // YS
            for h in range(nst):
                n = S * fqs * NIB
                # rows (i_loc, b), cols (f, j, tau)
                blk = flat[o:o + n].reshape(NI // NBLK, B, fqs, NBLK, S)
                y[:, :, :, :, c, f0:f0 + fqs] = \
                    blk.transpose(3, 0, 4, 1, 2)     # (j, i_loc, tau, b, f)
                o += n
                f0 += fqs
    return np.ascontiguousarray(y.reshape(T, B, F))


def kernel(x, weight, tail_padding):
    from concourse.bass_utils import run_bass_kernel_spmd

    nc = build_module()
    xs = prep_x(x)
    bs = prep_bands(weight)
    in_maps = [{"x": xs[c], "bands": bs[c]} for c in range(NCORES)]
    res = run_bass_kernel_spmd(nc, in_maps, list(range(NCORES)))
    shards = [res.results[c]["y"] for c in range(NCORES)]
    y = assemble_y(shards)
    seq_len = T if int(np.asarray(tail_padding)) else T - CTX
    return y[:seq_len]



# revision 4
# speedup vs baseline: 1.1092x; 1.1092x over previous
"""nn_Lookahead v9: flipped matmul (x stationary, bands moving), D=128.

Flip rationale: stationary loads are free in the cost model, so putting the
x time-tile slabs in the PE array and streaming the small band blocks as
the moving operand cuts PE to ~21us. That makes stride-128 tiles viable
again (no x overlap: -4.4us DMA) despite the spill matmul, since PE has
huge slack. Bands revert to the 3-region staging (A/B/C) at +1.8us.
DMA busy: x 23.3 + bands 7.65 + y 11.67 = 42.6us vs 45.2 for v8.

Per feature f, i-block j (8 tiles = 128 stationary columns):
  mA: psum[(i,b), tau 0:64]    = x[0:84, blk]^T   . bandA[0:84, 64]
  mB: psum[(i,b), tau 64:128]  = x[64:128, blk]^T . bandB[64:128@p64, 64]
  mC: psum[(ib<112), tau 64:128]+= x_next[0:20, blk+1]^T . bandC[0:20, 64]
band84[a,t] = w[f, a-t]*YGAIN (0<=a-t<=20); A = band84[0:84],
B = band84[0:64] restaged at p64..128, C = band84[64:84] at p0..20.
"""

import sys

sys.path.insert(0, "/opt/trn_rl_repo")

import numpy as np

T, B, F, K = 2048, 16, 1024, 21
YGAIN = 127.0 / 4.5
CTX = K - 1
NCORES = 8
FC = F // NCORES
S = 128            # time-tile size = stride (no overlap)
NI = T // S        # 16 tiles
NIB = NI * B       # 256 x-columns per feature
NBLK = 2           # i-blocks per feature (8 tiles = 128 stationary cols)
BLKC = NIB // NBLK  # 128
W64 = 64
AH = W64 + CTX     # 84
SB_B = W64         # bandB rows
CHUNKS = (12, 16, 24, 24, 24, 16, 8, 4)
YS = 2
HOLD_AT = 0
HOLD_CHUNKS = 4
HOLD_PRE = 5

assert sum(CHUNKS) == FC

_MODULE_CACHE = {}


def _offsets():
    xo, bo, yo = [], [], []
    brows = AH + SB_B + CTX   # 168 band rows per feature, 64 cols each
    x_acc = b_acc = y_acc = 0
    for fq in CHUNKS:
        xo.append(x_acc); x_acc += S * fq * NIB
        bo.append(b_acc); b_acc += brows * W64 * fq
        yo.append(y_acc); y_acc += S * fq * NIB
    return xo, bo, yo, x_acc, b_acc, y_acc


def build_module(repeat=1, bufs=(5, 3, 5, 8)):
    key = ("nc", repeat, bufs)
    if key in _MODULE_CACHE:
        return _MODULE_CACHE[key]
    import concourse.bacc as bacc
    import concourse.mybir as mybir
    from concourse.tile import TileContext

    xb, bb_, yb, pb = bufs
    dt = mybir.dt.float16
    dtx = mybir.dt.float8e3
    nc = bacc.Bacc("TRN2", target_bir_lowering=False, debug=False,
                   num_devices=NCORES)

    xo, bo, yo, xn, bn, yn = _offsets()
    x_d = nc.dram_tensor("x", [xn], dtx, kind="ExternalInput")
    b_d = nc.dram_tensor("bands", [bn], dt, kind="ExternalInput")
    y_d = nc.dram_tensor("y", [yn], mybir.dt.int8, kind="ExternalOutput")

    with TileContext(nc) as tc:
        with tc.tile_pool(name="xp", bufs=xb) as xp, \
             tc.tile_pool(name="bp", bufs=bb_) as bp, \
             tc.tile_pool(name="yp", bufs=yb) as yp, \
             tc.tile_pool(name="yh", bufs=2 * HOLD_CHUNKS) as yh, \
             tc.tile_pool(name="pp", bufs=pb, space="PSUM") as pp:
            for _ in range(repeat):
                held = []
                for ci, fq in enumerate(CHUNKS):
                    if ci == len(CHUNKS) - 1 and HOLD_PRE and held:
                        for hdst, hsb in held[:HOLD_PRE]:
                            nc.sync.dma_start(out=hdst, in_=hsb[:])
                        held = held[HOLD_PRE:]
                    fq2 = fq // YS
                    r1 = fq * W64
                    xq = xp.tile([S, fq * NIB], dtx, tag="x")
                    bb = bp.tile([S, 2 * fq * W64], dt, tag="bb")

                    x_src = x_d.ap()[xo[ci]:xo[ci] + S * fq * NIB] \
                        .rearrange("(s m) -> s m", s=S, m=fq * NIB)
                    nc.sync.dma_start(out=xq[:], in_=x_src)

                    ba = bo[ci]
                    a_n, b_n, c_n = AH * r1, SB_B * r1, CTX * r1
                    a_src = b_d.ap()[ba:ba + a_n] \
                        .rearrange("(a m) -> a m", a=AH, m=r1)
                    nc.sync.dma_start(out=bb[0:AH, 0:r1], in_=a_src)
                    b_src = b_d.ap()[ba + a_n:ba + a_n + b_n] \
                        .rearrange("(a m) -> a m", a=SB_B, m=r1)
                    nc.sync.dma_start(out=bb[W64:S, r1:2 * r1], in_=b_src)
                    c_src = b_d.ap()[ba + a_n + b_n:ba + a_n + b_n + c_n] \
                        .rearrange("(a m) -> a m", a=CTX, m=r1)
                    nc.sync.dma_start(out=bb[0:CTX, r1:2 * r1], in_=c_src)

                    last = ci == len(CHUNKS) - 1
                    ysb = None
                    for fi in range(fq):
                        if last and fi == HOLD_AT and held:
                            for hdst, hsb in held:
                                nc.sync.dma_start(out=hdst, in_=hsb[:])
                            held = []
                        if last:
                            if fi == 0:
                                ysb = yp.tile([S, fq * NIB], mybir.dt.int8,
                                              tag="y")
                        elif fi % fq2 == 0:
                            if ci < HOLD_CHUNKS:
                                ysb = yh.tile([S, fq2 * NIB], mybir.dt.int8,
                                              tag="yh")
                            else:
                                ysb = yp.tile([S, fq2 * NIB], mybir.dt.int8,
                                              tag="y")
                        pt = pp.tile([S, NIB], mybir.dt.float32, tag="ps")
                        wa = fi * W64
                        for j in range(NBLK):
                            cb = j * BLKC
                            xw = fi * NIB + j * BLKC
                            # mA: stationary x rows 0:84, moving bandA.
                            nc.tensor.matmul(
                                pt[0:S, cb:cb + W64],
                                lhsT=xq[0:AH, xw:xw + BLKC],
                                rhs=bb[0:AH, wa:wa + W64],
                                start=True, stop=True, skip_group_check=True)
                            # mB: stationary x rows 64:128, moving bandB.
                            nc.tensor.matmul(
                                pt[0:S, cb + W64:cb + BLKC],
                                lhsT=xq[W64:S, xw:xw + BLKC],
                                rhs=bb[W64:S, r1 + wa:r1 + wa + W64],
                                start=True, stop=False,
                                skip_group_check=True)
                            # mC: next-tile spill; block 1 drops tile 15
                            # (zero tail padding -> 7-tile stationary).
                            nc2 = BLKC if j == 0 else BLKC - B
                            nc.tensor.matmul(
                                pt[0:nc2, cb + W64:cb + BLKC],
                                lhsT=xq[0:CTX, xw + B:xw + B + nc2],
                                rhs=bb[0:CTX, r1 + wa:r1 + wa + W64],
                                start=False, stop=True,
                                skip_group_check=True)
                        fl = fi if last else fi % fq2
                        nhalf = fq if last else fq2
                        yc = fl * NIB
                        if (nhalf - 1 - fl) % 2 == 1:
                            nc.vector.tensor_copy(ysb[:, yc:yc + NIB],
                                                  pt[:, :])
                        else:
                            nc.scalar.copy(ysb[:, yc:yc + NIB], pt[:, :])
                        if not last and fi % fq2 == fq2 - 1:
                            h = fi // fq2
                            dst = y_d.ap()[yo[ci] + h * S * fq2 * NIB:
                                           yo[ci] + (h + 1) * S * fq2 * NIB] \
                                .rearrange("(s m) -> s m", s=S, m=fq2 * NIB)
                            if ci < HOLD_CHUNKS:
                                held.append((dst, ysb))
                            else:
                                nc.scalar.dma_start(out=dst, in_=ysb[:])
                    if last:
                        dst2 = y_d.ap()[yo[ci]:yo[ci] + S * fq * NIB] \
                            .rearrange("(s m) -> s m", s=S, m=fq * NIB)
                        nc.sync.dma_start(out=dst2, in_=ysb[:])
                for dst, ysb in held:
                    nc.scalar.dma_start(out=dst, in_=ysb[:])

    nc.compile()
    _MODULE_CACHE[key] = nc
    return nc


def prep_x(x):
    """x (T,B,F) -> per-core flat fp8e3 [s, f, i, b], 16 non-overlap tiles."""
    import ml_dtypes
    xr = np.asarray(x, dtype=np.float32).reshape(NI, S, B, NCORES, FC)
    out = []
    for c in range(NCORES):
        parts = []
        f0 = 0
        for fq in CHUNKS:
            blk = xr[:, :, :, c, f0:f0 + fq]          # (i, s, b, f)
            parts.append(np.ascontiguousarray(
                blk.transpose(1, 3, 0, 2)).ravel())   # (s, f, i, b)
            f0 += fq
        out.append(np.concatenate(parts).astype(ml_dtypes.float8_e3m4))
    return np.stack(out)


def prep_bands(weight):
    """weight (F,21) -> per-core flat band regions A/B/C, (a, f, t)."""
    w = np.asarray(weight, dtype=np.float32).reshape(NCORES, FC, K) * YGAIN
    band = np.zeros((NCORES, AH, FC, W64), np.float32)
    for k in range(K):
        for tt in range(W64):
            band[:, tt + k, :, tt] = w[:, :, k]
    out = []
    for c in range(NCORES):
        parts = []
        f0 = 0
        for fq in CHUNKS:
            blk = band[c, :, f0:f0 + fq, :]
            parts.append(blk[0:AH].ravel())
            parts.append(blk[0:SB_B].ravel())
            parts.append(blk[SB_B:AH].ravel())
            f0 += fq
        out.append(np.concatenate(parts).astype(np.float16))
    return np.stack(out)


def assemble_y(shards):
    """per-core flat int8 y [(i_loc,b), (f, j, tau)] -> (T,B,F) fp32."""
    y = np.empty((NBLK, NI // NBLK, S, B, NCORES, FC), np.float32)
    for c in range(NCORES):
        flat = np.asarray(shards[c]).astype(np.float32).ravel() / YGAIN
        f0 = 0
        o = 0
        for ci, fq in enumerate(CHUNKS):
            lastc = ci == len(CHUNKS) - 1
            nst = 1 if lastc else YS
            fqs = fq if lastc else fq // YS
            for h in range(nst):
                n = S * fqs * NIB
                # rows (i_loc, b), cols (f, j, tau)
                blk = flat[o:o + n].reshape(NI // NBLK, B, fqs, NBLK, S)
                y[:, :, :, :, c, f0:f0 + fqs] = \
                    blk.transpose(3, 0, 4, 1, 2)     # (j, i_loc, tau, b, f)
                o += n
                f0 += fqs
    return np.ascontiguousarray(y.reshape(T, B, F))


def kernel(x, weight, tail_padding):
    from concourse.bass_utils import run_bass_kernel_spmd

    nc = build_module()
    xs = prep_x(x)
    bs = prep_bands(weight)
    in_maps = [{"x": xs[c], "bands": bs[c]} for c in range(NCORES)]
    res = run_bass_kernel_spmd(nc, in_maps, list(range(NCORES)))
    shards = [res.results[c]["y"] for c in range(NCORES)]
    y = assemble_y(shards)
    seq_len = T if int(np.asarray(tail_padding)) else T - CTX
    return y[:seq_len]

